# revision 15
# baseline (speedup 1.0000x reference)
"""ALBEF concept-text contrastive loss on 8 TRN2 NeuronCores.

Key algebraic facts used (verified vs the jax reference to 3e-8):
  * sim_t2i == sim_i2t.T exactly, so only one [B, B] similarity matrix is
    needed; the loss is the mean of row- and column- log-softmax diagonals.
  * sim_i2t[i,j] = term_col + term_row, with S[i,j,q,l] = cf_hat[i,q]. wf_hat[j,l]:
      term_col = sum_q  max_{valid l} S / (Q * temp)
      term_row = sum_{valid l} max_q S / (nw[j] * temp)
  * Invalid text positions (CLS/SEP/pad) are replaced on the host by copies of
    position 1 (always valid), which makes max over ALL l == max over valid l.
    The masked l-sum is folded into a small block-diagonal "Emask" matmul whose
    weights are mask[j,l]/(nw[j]*temp); the q-sum is an "Eones" matmul.

Per-core dataflow (core r owns batch rows r*32:(r+1)*32):
  stage 1/2: project+normalize local concept/text shards, PE-transpose to
    [D, rows] layout (cfT columns are q-major: col = q*32 + i).
  stage 3: AllGather of projected text features, pipelined in 3 column chunks
    so pass A/B compute starts when the first chunk lands.
  pass A: S chunks [part=(q,i), free=(j,l)]; DVE segmented max over l ->
    colmax; Eones matmul accumulates sum_q into term_col (PSUM).
  pass B: S chunks [part=(j,l), free=(q,i) q-major]; ACT copies PSUM->SBUF
    bf16; GpSimd computes max over q by 5 contiguous halving tensor-max ops;
    Emask matmul accumulates the masked l-sum into term_row (PSUM).
  stage 6: sim rows [32, 256] = term_col + term_row^T.
  stage 7: AllGather sim; per-core log-softmax loss (row + col) -> scalar.
"""

import ml_dtypes
import numpy as np

import concourse.bass as bass
import concourse.bacc as bacc
import concourse.mybir as mybir
import concourse.tile as tile
from concourse.bass_utils import run_bass_kernel_spmd

F32 = mybir.dt.float32
BF16 = mybir.dt.bfloat16
AX = mybir.AxisListType
ALU = mybir.AluOpType
ACTF = mybir.ActivationFunctionType

B, Q, L, VW, TW, D = 256, 32, 40, 768, 768, 256
NCORES = 8
BL = B // NCORES            # 32 local batch rows
IQ = BL * Q                 # 1024 local (q,i) rows
JLL = BL * L                # 1280 local (j,l) rows
JL = B * L                  # 10240 global (j,l)
KC = VW // 128              # 6 contraction chunks for projection
NMC_A = IQ // 128           # 8 M-chunks in pass A
# column groups (for AG pipelining + pass chunking): (offset, width, j0, nj)
CGROUPS = [(0, 480, 0, 12), (480, 480, 12, 12), (960, 320, 24, 8)]
# pass B chunk widths within each column group
CG_BCHUNKS = {0: [120, 120, 120, 120], 1: [120, 120, 120, 120], 2: [120, 120, 80]}
NB = sum(len(v) for v in CG_BCHUNKS.values()) * NCORES  # 88

_CACHE = {}


def _build():
    nc = _build_graph()
    nc.compile()
    return nc


def _build_graph():
    nc = bacc.Bacc("TRN2", target_bir_lowering=False, debug=False,
                   num_devices=NCORES)

    concept_t = nc.dram_tensor("concept_t", [VW, IQ], BF16, kind="ExternalInput")
    text_t = nc.dram_tensor("text_t", [TW, JLL], BF16, kind="ExternalInput")
    wc = nc.dram_tensor("wc", [VW, D], BF16, kind="ExternalInput")
    ww = nc.dram_tensor("ww", [TW, D], BF16, kind="ExternalInput")
    bcb = nc.dram_tensor("bcb", [128, D], F32, kind="ExternalInput")
    bwb = nc.dram_tensor("bwb", [128, D], F32, kind="ExternalInput")
    ident = nc.dram_tensor("ident", [128, 128], F32, kind="ExternalInput")
    eones = nc.dram_tensor("eones", [128, BL], BF16, kind="ExternalInput")
    emaskt = nc.dram_tensor("emaskt", [NB, 128, 128], BF16, kind="ExternalInput")
    dmask = nc.dram_tensor("dmask", [2, 128, B], F32, kind="ExternalInput")
    ones128 = nc.dram_tensor("ones128", [128, 1], F32, kind="ExternalInput")

    out = nc.dram_tensor("out", [1, 1], F32, kind="ExternalOutput")

    with tile.TileContext(nc) as tc:
        with (
            tc.tile_pool(name="cst", bufs=1) as cst,
            tc.tile_pool(name="feat", bufs=1) as feat,
            tc.tile_pool(name="pterm", bufs=1, space="PSUM") as pterm,
            tc.tile_pool(name="dram", bufs=1, space="DRAM") as dram,
        ):
            # ---- persistent SBUF tiles ----
            cfT = [feat.tile([128, IQ], BF16, tag=f"cfT{k}", name=f"cfT{k}")
                   for k in range(2)]
            wfl = [feat.tile([128, JLL], BF16, tag=f"wfl{k}", name=f"wfl{k}")
                   for k in range(2)]
            sim_sb = feat.tile([BL, B], F32, tag="sim_sb")

            emask_sb = cst.tile([128, NB * 128], BF16, tag="emask_sb")
            eones_sb = cst.tile([128, BL], BF16, tag="eones_sb")
            ident_sb = cst.tile([128, 128], F32, tag="ident_sb")
            bcb_sb = cst.tile([128, D], F32, tag="bcb_sb")
            bwb_sb = cst.tile([128, D], F32, tag="bwb_sb")
            dmask_sb = cst.tile([128, 2 * B], F32, tag="dmask_sb")
            ones_sb = cst.tile([128, 1], F32, tag="ones_sb")

            # one PSUM bank shared by all three accumulation regions
            pterm_t = pterm.tile([128, 512], F32, tag="pterm_t")
            term_col = pterm_t[:, 0:B]
            term_row = [pterm_t[:, B + h * BL: B + (h + 1) * BL] for h in range(2)]

            nc.sync.dma_start(emask_sb[:].rearrange("p (c m) -> p c m", m=128),
                              emaskt[:].rearrange("c p m -> p c m"))
            nc.sync.dma_start(eones_sb[:], eones[:])
            nc.sync.dma_start(ident_sb[:], ident[:])
            nc.sync.dma_start(bcb_sb[:], bcb[:])
            nc.sync.dma_start(bwb_sb[:], bwb[:])
            nc.sync.dma_start(dmask_sb[:].rearrange("p (t j) -> p t j", j=B),
                              dmask[:].rearrange("t p j -> p t j"))
            nc.sync.dma_start(ones_sb[:], ones128[:])

            # ---- stage 1/2: projections + l2norm + transpose ----
            with (
                tc.tile_pool(name="pin", bufs=1) as pin,
                tc.tile_pool(name="ps2", bufs=2, space="PSUM") as ps2,
                tc.tile_pool(name="wk2", bufs=2) as wk2,
            ):
                cin = pin.tile([128, KC * IQ], BF16, tag="cin")
                tin = pin.tile([128, KC * JLL], BF16, tag="tin")
                wcs = pin.tile([128, KC * D], BF16, tag="wcs")
                wws = pin.tile([128, KC * D], BF16, tag="wws")
                for k in range(KC):
                    nc.sync.dma_start(tin[:, k * JLL:(k + 1) * JLL],
                                      text_t[k * 128:(k + 1) * 128, :])
                    nc.sync.dma_start(cin[:, k * IQ:(k + 1) * IQ],
                                      concept_t[k * 128:(k + 1) * 128, :])
                    nc.sync.dma_start(wcs[:, k * D:(k + 1) * D],
                                      wc[k * 128:(k + 1) * 128, :])
                    nc.sync.dma_start(wws[:, k * D:(k + 1) * D],
                                      ww[k * 128:(k + 1) * 128, :])

                def project(src, width, w_sb, bias_sb, dstT):
                    for m in range(width // 128):
                        pp = ps2.tile([128, D], F32, tag="pp")
                        for k in range(KC):
                            nc.tensor.matmul(
                                pp[:],
                                lhsT=src[:, k * width + m * 128:
                                         k * width + (m + 1) * 128],
                                rhs=w_sb[:, k * D:(k + 1) * D],
                                start=(k == 0), stop=(k == KC - 1))
                        t_sb = wk2.tile([128, D], F32, tag="t_sb")
                        nc.vector.scalar_tensor_tensor(
                            t_sb[:], pp[:], 1.0, bias_sb[:],
                            op0=ALU.mult, op1=ALU.add)
                        sq = wk2.tile([128, D], F32, tag="sq")
                        ss = wk2.tile([128, 1], F32, tag="ss")
                        nc.scalar.activation(sq[:], t_sb[:], ACTF.Square,
                                             accum_out=ss[:])
                        ssq = wk2.tile([128, 1], F32, tag="ssq")
                        nc.scalar.sqrt(ssq[:], ss[:])
                        rn = wk2.tile([128, 1], F32, tag="rn")
                        nc.vector.reciprocal(rn[:], ssq[:])
                        nsb = wk2.tile([128, D], F32, tag="nsb")
                        nc.vector.tensor_scalar_mul(nsb[:], t_sb[:], rn[:])
                        for kk in range(2):
                            ptr = ps2.tile([128, 128], F32, tag="ptr")
                            nc.tensor.transpose(
                                ptr[:], nsb[:, kk * 128:(kk + 1) * 128],
                                ident_sb[:])
                            nc.scalar.copy(
                                dstT[kk][:, m * 128:(m + 1) * 128], ptr[:])

                project(tin, JLL, wws, bwb_sb, wfl)
                project(cin, IQ, wcs, bcb_sb, cfT)

            # ---- stages 3-7 ----
            with tc.tile_pool(name="feat2", bufs=1) as feat2:
                # wfg column-group tiles: [k][cg] -> [128, 8 * w] (rank-major)
                wfg = [[feat2.tile([128, NCORES * CGROUPS[cg][1]], BF16,
                                   tag=f"wfg{k}_{cg}", name=f"wfg{k}_{cg}")
                        for cg in range(3)]
                       for k in range(2)]

                # chunked AllGather of wfl column groups
                for cg, (off, w, _, _) in enumerate(CGROUPS):
                    ag_in = dram.tile([2 * 128, w], BF16, name=f"ag_in{cg}")
                    ag_out = dram.tile([NCORES * 2 * 128, w], BF16,
                                       name=f"ag_out{cg}")
                    for k in range(2):
                        nc.sync.dma_start(ag_in[k * 128:(k + 1) * 128, :],
                                          wfl[k][:, off:off + w])
                    nc.gpsimd.collective_compute(
                        "AllGather", ALU.bypass,
                        ins=[ag_in.opt()], outs=[ag_out.opt()],
                        replica_groups=[list(range(NCORES))])
                    for rr in range(NCORES):
                        for k in range(2):
                            nc.sync.dma_start(
                                wfg[k][cg][:, rr * w:(rr + 1) * w],
                                ag_out[rr * 256 + k * 128:
                                       rr * 256 + (k + 1) * 128, :])

                with (
                    tc.tile_pool(name="psa", bufs=3, space="PSUM") as psa,
                    tc.tile_pool(name="cmx", bufs=1) as cmx,
                    tc.tile_pool(name="psb", bufs=2, space="PSUM") as psb,
                    tc.tile_pool(name="wkb", bufs=3) as wkb,
                ):
                    # ---- pass A (term_col), pipelined over column groups ----
                    colmax = [cmx.tile([128, B], BF16, tag=f"colmax{m}",
                                       name=f"colmax{m}") for m in range(NMC_A)]
                    for cg, (off, w, j0, nj) in enumerate(CGROUPS):
                        for m in range(NMC_A):
                            for rr in range(NCORES):
                                pa = psa.tile([128, 512], F32, tag="pa")
                                for k in range(2):
                                    mm = nc.tensor.matmul(
                                        pa[:, 0:w],
                                        lhsT=cfT[k][:, m * 128:(m + 1) * 128],
                                        rhs=wfg[k][cg][:, rr * w:(rr + 1) * w],
                                        start=(k == 0), stop=(k == 1))
                                nc.vector.reduce_max(
                                    colmax[m][:, rr * BL + j0: rr * BL + j0 + nj],
                                    pa[:, 0:w].rearrange("p (j l) -> p j l", l=L),
                                    axis=AX.X)
                    for m in range(NMC_A):
                        nc.tensor.matmul(
                            term_col[0:BL, :],
                            lhsT=eones_sb[:],
                            rhs=colmax[m][:],
                            start=(m == 0), stop=(m == NMC_A - 1))

                    # ---- pass B (term_row) ----
                    cc = 0
                    for cg, (off, w, j0, nj) in enumerate(CGROUPS):
                        for rr in range(NCORES):
                            h = rr // 4
                            cbase = 0
                            for ci, P in enumerate(CG_BCHUNKS[cg]):
                                pb = psb.tile([128, IQ], F32, tag="pb")
                                for k in range(2):
                                    for n in range(2):
                                        mm = nc.tensor.matmul(
                                            pb[0:P, n * 512:(n + 1) * 512],
                                            lhsT=wfg[k][cg][:, rr * w + cbase:
                                                            rr * w + cbase + P],
                                            rhs=cfT[k][:, n * 512:(n + 1) * 512],
                                            start=(k == 0), stop=(k == 1))
                                pbs = wkb.tile([128, IQ], BF16, tag="pbs")
                                nc.scalar.copy(pbs[0:P, :], pb[0:P, :])
                                t1 = wkb.tile([128, 512], BF16, tag="t1")
                                nc.vector.tensor_tensor(
                                    t1[0:P, :], pbs[0:P, 0:512],
                                    pbs[0:P, 512:1024], op=ALU.max)
                                t2 = wkb.tile([128, 256], BF16, tag="t2")
                                nc.vector.tensor_tensor(
                                    t2[0:P, :], t1[0:P, 0:256], t1[0:P, 256:512],
                                    op=ALU.max)
                                t3 = wkb.tile([128, 128], BF16, tag="t3")
                                nc.vector.tensor_tensor(
                                    t3[0:P, :], t2[0:P, 0:128], t2[0:P, 128:256],
                                    op=ALU.max)
                                t4 = wkb.tile([128, 64], BF16, tag="t4")
                                nc.vector.tensor_tensor(
                                    t4[0:P, :], t3[0:P, 0:64], t3[0:P, 64:128],
                                    op=ALU.max)
                                rmt = wkb.tile([128, BL], BF16, tag="rmt")
                                nc.vector.tensor_tensor(
                                    rmt[0:P, :], t4[0:P, 0:32], t4[0:P, 32:64],
                                    op=ALU.max)
                                nc.tensor.matmul(
                                    term_row[h],
                                    lhsT=emask_sb[0:P, cc * 128:(cc + 1) * 128],
                                    rhs=rmt[0:P, :],
                                    start=(cg == 0 and rr % 4 == 0 and ci == 0),
                                    stop=(cg == 2 and rr % 4 == 3 and ci == 2))
                                cbase += P
                                cc += 1

                # ---- stage 6: sim rows = term_col + term_row^T ----
                with (
                    tc.tile_pool(name="ps6", bufs=2, space="PSUM") as ps6,
                    tc.tile_pool(name="wk6", bufs=2) as wk6,
                ):
                    for h in range(2):
                        trs = wk6.tile([128, BL], F32, tag="trs")
                        nc.scalar.copy(trs[:], term_row[h])
                        ptt = ps6.tile([BL, 128], F32, tag="ptt")
                        nc.tensor.transpose(ptt[:], trs[:], ident_sb[:])
                        tts = wk6.tile([BL, 128], F32, tag="tts")
                        nc.scalar.copy(tts[:], ptt[:])
                        nc.vector.tensor_tensor(
                            sim_sb[:, h * 128:(h + 1) * 128],
                            term_col[0:BL, h * 128:(h + 1) * 128],
                            tts[:], op=ALU.add)

                # ---- stage 7: AllGather sim + loss ----
                ag2_in = dram.tile([BL, B], F32)
                ag2_out = dram.tile([B, B], F32)
                nc.sync.dma_start(ag2_in[:], sim_sb[:])
                nc.gpsimd.collective_compute(
                    "AllGather", ALU.bypass,
                    ins=[ag2_in.opt()], outs=[ag2_out.opt()],
                    replica_groups=[list(range(NCORES))])

                with (
                    tc.tile_pool(name="ps7", bufs=2, space="PSUM") as ps7,
                    tc.tile_pool(name="wk7", bufs=1) as wk7,
                ):
                    simf = []
                    for t in range(2):
                        sf = wk7.tile([128, B], F32, tag=f"simf{t}",
                                      name=f"simf{t}")
                        nc.sync.dma_start(sf[:], ag2_out[t * 128:(t + 1) * 128, :])
                        simf.append(sf)

                    loss_acc = wk7.tile([128, 4], F32, tag="loss_acc")

                    def lse_diag_col(src_tile, t, col):
                        nrmax = wk7.tile([128, 1], F32, tag="nrmax")
                        nc.vector.tensor_reduce(nrmax[:], src_tile[:], axis=AX.X,
                                                op=ALU.max, negate=True)
                        escr = wk7.tile([128, B], F32, tag="escr")
                        sume = wk7.tile([128, 1], F32, tag="sume")
                        nc.scalar.activation(escr[:], src_tile[:], ACTF.Exp,
                                             bias=nrmax[:], scale=1.0,
                                             accum_out=sume[:])
                        lg = wk7.tile([128, 1], F32, tag="lg")
                        nc.scalar.activation(lg[:], sume[:], ACTF.Ln)
                        dscr = wk7.tile([128, B], F32, tag="dscr")
                        dg = wk7.tile([128, 1], F32, tag="dg")
                        nc.vector.scalar_tensor_tensor(
                            dscr[:], src_tile[:], 1.0,
                            dmask_sb[:, t * B:(t + 1) * B],
                            op0=ALU.mult, op1=ALU.mult, accum_out=dg[:])
                        nc.vector.scalar_tensor_tensor(
                            loss_acc[:, col:col + 1], nrmax[:], dg[:], lg[:],
                            op0=ALU.add, op1=ALU.subtract)

                    for t in range(2):
                        lse_diag_col(simf[t], t, t)

                    for t in range(2):
                        sT = wk7.tile([128, B], F32, tag=f"simT{t}",
                                      name=f"simT{t}")
                        for u in range(2):
                            pt = ps7.tile([128, 128], F32, tag="pt7")
                            nc.tensor.transpose(
                                pt[:], simf[u][:, t * 128:(t + 1) * 128],
                                ident_sb[:])
                            nc.scalar.copy(sT[:, u * 128:(u + 1) * 128], pt[:])
                        lse_diag_col(sT, t, 2 + t)

                    fin = ps7.tile([1, 4], F32, tag="fin")
                    nc.tensor.matmul(fin[0:1, :], lhsT=ones_sb[:],
                                     rhs=loss_acc[:], start=True, stop=True)
                    red = wk7.tile([1, 1], F32, tag="red")
                    nc.vector.reduce_sum(red[0:1, :], fin[0:1, :], axis=AX.X)
                    osb = wk7.tile([1, 1], F32, tag="osb")
                    nc.scalar.mul(osb[0:1, :], red[0:1, :], -1.0 / (2 * B))
                    nc.sync.dma_start(out[:], osb[0:1, :])

    return nc


def _host_prep(inputs):
    concept_feat = np.ascontiguousarray(np.asarray(inputs["concept_feat"],
                                                   dtype=np.float32))
    text_embeds = np.array(np.asarray(inputs["text_embeds"],
                                      dtype=np.float32), copy=True)
    text_mask = np.asarray(inputs["text_mask"]).astype(np.int32)
    Wc = np.ascontiguousarray(np.asarray(inputs["Wc"], dtype=np.float32))
    bc = np.asarray(inputs["bc"], dtype=np.float32)
    Ww = np.ascontiguousarray(np.asarray(inputs["Ww"], dtype=np.float32))
    bw = np.asarray(inputs["bw"], dtype=np.float32)
    temp = float(np.asarray(inputs["temp_cpt"]))

    # word mask (drop CLS + SEP), valid counts
    m = text_mask.copy()
    m[:, 0] = 0
    sep = (L - 1) - np.argmax(m[:, ::-1] > 0, axis=1)
    m[np.arange(B), sep] = 0
    nw = m.sum(axis=1).astype(np.float32)

    # sanitize invalid text rows with copies of position 1 (always valid)
    for j in range(B):
        inv = m[j] == 0
        text_embeds[j, inv] = text_embeds[j, 1]

    # Eones: partition p of any pass-A M-chunk maps to i = p % BL
    eones = np.zeros((128, BL), dtype=np.float32)
    eones[np.arange(128), np.arange(128) % BL] = 1.0 / (Q * temp)

    # Emask tiles: [NB, 128, 128], chunk order must match pass B loops
    emaskt = np.zeros((NB, 128, 128), dtype=np.float32)
    cc = 0
    for cg, (off, wdt, j0, nj) in enumerate(CGROUPS):
        for rr in range(NCORES):
            cbase = off
            for P in CG_BCHUNKS[cg]:
                for p in range(P):
                    jl = cbase + p
                    j_loc, l = divmod(jl, L)
                    jg = rr * BL + j_loc
                    if m[jg, l]:
                        emaskt[cc, p, jg % 128] = 1.0 / (nw[jg] * temp)
                cbase += P
                cc += 1

    dmask = np.zeros((2, 128, B), dtype=np.float32)
    for t in range(2):
        dmask[t, np.arange(128), t * 128 + np.arange(128)] = 1.0

    shared = {
        "wc": Wc.astype(ml_dtypes.bfloat16), "ww": Ww.astype(ml_dtypes.bfloat16),
        "bcb": np.tile(bc[None, :], (128, 1)).astype(np.float32),
        "bwb": np.tile(bw[None, :], (128, 1)).astype(np.float32),
        "ident": np.eye(128, dtype=np.float32),
        "eones": eones.astype(ml_dtypes.bfloat16),
        "emaskt": emaskt.astype(ml_dtypes.bfloat16),
        "dmask": dmask,
        "ones128": np.ones((128, 1), dtype=np.float32),
    }
    in_maps = []
    for r in range(NCORES):
        im = dict(shared)
        # cfT columns are q-major: row order (q, i)
        im["concept_t"] = np.ascontiguousarray(
            concept_feat[r * BL:(r + 1) * BL].transpose(1, 0, 2)
            .reshape(IQ, VW).T).astype(ml_dtypes.bfloat16)
        im["text_t"] = np.ascontiguousarray(
            text_embeds[r * BL:(r + 1) * BL].reshape(JLL, TW).T
        ).astype(ml_dtypes.bfloat16)
        in_maps.append(im)
    return in_maps


def kernel(**inputs):
    in_maps = _host_prep(inputs)
    if "nc" not in _CACHE:
        _CACHE["nc"] = _build()
    res = run_bass_kernel_spmd(_CACHE["nc"], in_maps,
                               core_ids=list(range(NCORES)))
    return np.float32(res.results[0]["out"][0, 0])


# revision 21
# speedup vs baseline: 1.2254x; 1.2254x over previous
"""ALBEF concept-text contrastive loss on 8 TRN2 NeuronCores.

Key algebraic facts used (verified vs the jax reference to 3e-8):
  * sim_t2i == sim_i2t.T exactly, so only one [B, B] similarity matrix is
    needed; the loss is the mean of row- and column- log-softmax diagonals.
  * sim_i2t[i,j] = term_col + term_row, with S[i,j,q,l] = cf_hat[i,q]. wf_hat[j,l]:
      term_col = sum_q  max_{valid l} S / (Q * temp)
      term_row = sum_{valid l} max_q S / (nw[j] * temp)
  * Invalid text positions (CLS/SEP/pad) are replaced on the host by copies of
    position 1 (always valid), which makes max over ALL l == max over valid l.
    The masked l-sum is folded into a small block-diagonal "Emask" matmul whose
    weights are mask[j,l]/(nw[j]*temp); the q-sum is an "Eones" matmul.

Per-core dataflow (core r owns batch rows r*32:(r+1)*32):
  stage 1/2: project+normalize local concept/text shards, PE-transpose to
    [D, rows] layout (cfT columns are q-major: col = q*32 + i).
  stage 3: AllGather of projected text features, pipelined in 3 column chunks
    so pass A/B compute starts when the first chunk lands.
  pass A: S chunks [part=(q,i), free=(j,l)]; DVE segmented max over l ->
    colmax; Eones matmul accumulates sum_q into term_col (PSUM).
  pass B: S chunks [part=(j,l), free=(q,i) q-major]; ACT copies PSUM->SBUF
    bf16; GpSimd computes max over q by 5 contiguous halving tensor-max ops;
    Emask matmul accumulates the masked l-sum into term_row (PSUM).
  stage 6: sim rows [32, 256] = term_col + term_row^T.
  stage 7: AllGather sim; per-core log-softmax loss (row + col) -> scalar.
"""

import ml_dtypes
import numpy as np

import concourse.bass as bass
import concourse.bacc as bacc
import concourse.mybir as mybir
import concourse.tile as tile
from concourse.bass_utils import run_bass_kernel_spmd

F32 = mybir.dt.float32
BF16 = mybir.dt.bfloat16
AX = mybir.AxisListType
ALU = mybir.AluOpType
ACTF = mybir.ActivationFunctionType

B, Q, L, VW, TW, D = 256, 32, 40, 768, 768, 256
NCORES = 8
BL = B // NCORES            # 32 local batch rows
IQ = BL * Q                 # 1024 local (q,i) rows
JLL = BL * L                # 1280 local (j,l) rows
JL = B * L                  # 10240 global (j,l)
KC = VW // 128              # 6 contraction chunks for projection
NMC_A = IQ // 128           # 8 M-chunks in pass A
# column groups (for AG pipelining + pass chunking): (offset, width, j0, nj)
CGROUPS = [(0, 480, 0, 12), (480, 480, 12, 12), (960, 320, 24, 8)]
# pass B chunk widths within each column group
CG_BCHUNKS = {0: [120, 120, 120, 120], 1: [120, 120, 120, 120], 2: [120, 120, 80]}
NB = sum(len(v) for v in CG_BCHUNKS.values()) * NCORES  # 88

_CACHE = {}


def _build():
    nc = _build_graph()
    nc.compile()
    return nc


def _build_graph():
    nc = bacc.Bacc("TRN2", target_bir_lowering=False, debug=False,
                   num_devices=NCORES)

    concept_t = nc.dram_tensor("concept_t", [VW, IQ], BF16, kind="ExternalInput")
    text_t = nc.dram_tensor("text_t", [TW, JLL], BF16, kind="ExternalInput")
    wc = nc.dram_tensor("wc", [VW, D], BF16, kind="ExternalInput")
    ww = nc.dram_tensor("ww", [TW, D], BF16, kind="ExternalInput")
    bcb = nc.dram_tensor("bcb", [128, D], F32, kind="ExternalInput")
    bwb = nc.dram_tensor("bwb", [128, D], F32, kind="ExternalInput")
    ident = nc.dram_tensor("ident", [128, 128], F32, kind="ExternalInput")
    eones = nc.dram_tensor("eones", [128, BL], BF16, kind="ExternalInput")
    emaskt = nc.dram_tensor("emaskt", [NB, 128, 128], BF16, kind="ExternalInput")
    dmask = nc.dram_tensor("dmask", [2, 128, B], F32, kind="ExternalInput")
    ones128 = nc.dram_tensor("ones128", [128, 1], F32, kind="ExternalInput")

    out = nc.dram_tensor("out", [1, 1], F32, kind="ExternalOutput")
    import os
    dbg = None
    if os.environ.get("KDBG"):
        dbg = nc.dram_tensor("dbg", [B, B], F32, kind="ExternalOutput")
    dbg3 = None
    if os.environ.get("KDBG3"):
        dbg3 = nc.dram_tensor("dbg3", [128, 320], F32, kind="ExternalOutput")
    dbg2 = None
    if os.environ.get("KDBG2"):
        dbg2 = nc.dram_tensor("dbg2", [128, NCORES * 480], F32, kind="ExternalOutput")

    with tile.TileContext(nc) as tc:
        with (
            tc.tile_pool(name="cst", bufs=1) as cst,
            tc.tile_pool(name="feat", bufs=1) as feat,
            tc.tile_pool(name="pterm", bufs=1, space="PSUM") as pterm,
            tc.tile_pool(name="dram", bufs=1, space="DRAM") as dram,
        ):
            # ---- persistent SBUF tiles ----
            cfT = [feat.tile([128, IQ], BF16, tag=f"cfT{k}", name=f"cfT{k}")
                   for k in range(2)]
            wfl = [feat.tile([128, JLL], BF16, tag=f"wfl{k}", name=f"wfl{k}")
                   for k in range(2)]
            sim_sb = feat.tile([BL, B], F32, tag="sim_sb")

            emask_sb = cst.tile([128, NB * 128], BF16, tag="emask_sb")
            eones_sb = cst.tile([128, BL], BF16, tag="eones_sb")
            ident_sb = cst.tile([128, 128], F32, tag="ident_sb")
            bcb_sb = cst.tile([128, D], F32, tag="bcb_sb")
            bwb_sb = cst.tile([128, D], F32, tag="bwb_sb")
            dmask_sb = cst.tile([128, 2 * B], F32, tag="dmask_sb")
            ones_sb = cst.tile([128, 1], F32, tag="ones_sb")

            term_col = pterm.tile([BL, B], F32, tag="tcol")
            term_row = [pterm.tile([128, BL], F32, tag=f"trow{h}", name=f"trow{h}")
                        for h in range(2)]

            nc.sync.dma_start(emask_sb[:].rearrange("p (c m) -> p c m", m=128),
                              emaskt[:].rearrange("c p m -> p c m"))
            nc.sync.dma_start(eones_sb[:], eones[:])
            nc.sync.dma_start(ident_sb[:], ident[:])
            nc.sync.dma_start(bcb_sb[:], bcb[:])
            nc.sync.dma_start(bwb_sb[:], bwb[:])
            nc.sync.dma_start(dmask_sb[:].rearrange("p (t j) -> p t j", j=B),
                              dmask[:].rearrange("t p j -> p t j"))
            nc.sync.dma_start(ones_sb[:], ones128[:])

            # ---- stage 1/2: projections + l2norm + transpose ----
            with (
                tc.tile_pool(name="pin", bufs=1) as pin,
                tc.tile_pool(name="ps2", bufs=2, space="PSUM") as ps2,
                tc.tile_pool(name="wk2", bufs=2) as wk2,
            ):
                cin = pin.tile([128, KC * IQ], BF16, tag="cin")
                tin = pin.tile([128, KC * JLL], BF16, tag="tin")
                wcs = pin.tile([128, KC * D], BF16, tag="wcs")
                wws = pin.tile([128, KC * D], BF16, tag="wws")
                for k in range(KC):
                    nc.sync.dma_start(tin[:, k * JLL:(k + 1) * JLL],
                                      text_t[k * 128:(k + 1) * 128, :])
                    nc.sync.dma_start(cin[:, k * IQ:(k + 1) * IQ],
                                      concept_t[k * 128:(k + 1) * 128, :])
                    nc.sync.dma_start(wcs[:, k * D:(k + 1) * D],
                                      wc[k * 128:(k + 1) * 128, :])
                    nc.sync.dma_start(wws[:, k * D:(k + 1) * D],
                                      ww[k * 128:(k + 1) * 128, :])

                def project(src, width, w_sb, bias_sb, dstT):
                    for m in range(width // 128):
                        pp = ps2.tile([128, D], F32, tag="pp")
                        for k in range(KC):
                            nc.tensor.matmul(
                                pp[:],
                                lhsT=src[:, k * width + m * 128:
                                         k * width + (m + 1) * 128],
                                rhs=w_sb[:, k * D:(k + 1) * D],
                                start=(k == 0), stop=(k == KC - 1))
                        t_sb = wk2.tile([128, D], F32, tag="t_sb")
                        nc.vector.scalar_tensor_tensor(
                            t_sb[:], pp[:], 1.0, bias_sb[:],
                            op0=ALU.mult, op1=ALU.add)
                        sq = wk2.tile([128, D], F32, tag="sq")
                        ss = wk2.tile([128, 1], F32, tag="ss")
                        nc.scalar.activation(sq[:], t_sb[:], ACTF.Square,
                                             accum_out=ss[:])
                        ssq = wk2.tile([128, 1], F32, tag="ssq")
                        nc.scalar.sqrt(ssq[:], ss[:])
                        rn = wk2.tile([128, 1], F32, tag="rn")
                        nc.vector.reciprocal(rn[:], ssq[:])
                        nsb = wk2.tile([128, D], F32, tag="nsb")
                        nc.vector.tensor_scalar_mul(nsb[:], t_sb[:], rn[:])
                        for kk in range(2):
                            ptr = ps2.tile([128, 128], F32, tag="ptr")
                            nc.tensor.transpose(
                                ptr[:], nsb[:, kk * 128:(kk + 1) * 128],
                                ident_sb[:])
                            nc.scalar.copy(
                                dstT[kk][:, m * 128:(m + 1) * 128], ptr[:])

                project(tin, JLL, wws, bwb_sb, wfl)
                project(cin, IQ, wcs, bcb_sb, cfT)

            # ---- stages 3-7 ----
            with tc.tile_pool(name="feat2", bufs=1) as feat2:
                # wfg column-group tiles: [k][cg] -> [128, 8 * w] (rank-major)
                wfg = [[feat2.tile([128, NCORES * CGROUPS[cg][1]], BF16,
                                   tag=f"wfg{k}_{cg}", name=f"wfg{k}_{cg}")
                        for cg in range(3)]
                       for k in range(2)]

                # chunked AllGather of wfl column groups
                for cg, (off, w, _, _) in enumerate(CGROUPS):
                    ag_in = dram.tile([2 * 128, w], BF16, name=f"ag_in{cg}")
                    ag_out = dram.tile([NCORES * 2 * 128, w], BF16,
                                       name=f"ag_out{cg}")
                    for k in range(2):
                        nc.sync.dma_start(ag_in[k * 128:(k + 1) * 128, :],
                                          wfl[k][:, off:off + w])
                    nc.gpsimd.collective_compute(
                        "AllGather", ALU.bypass,
                        ins=[ag_in.opt()], outs=[ag_out.opt()],
                        replica_groups=[list(range(NCORES))])
                    for rr in range(NCORES):
                        for k in range(2):
                            nc.sync.dma_start(
                                wfg[k][cg][:, rr * w:(rr + 1) * w],
                                ag_out[rr * 256 + k * 128:
                                       rr * 256 + (k + 1) * 128, :])

                if dbg2 is not None:
                    with tc.tile_pool(name="dbgp", bufs=1) as dbgp:
                        dcp2 = dbgp.tile([128, NCORES * 480], F32, tag="dcp2")
                        nc.vector.tensor_copy(dcp2[:], wfg[0][0][:])
                        nc.sync.dma_start(dbg2[:], dcp2[:])
                with (
                    tc.tile_pool(name="psa", bufs=2, space="PSUM") as psa,
                    tc.tile_pool(name="cmx", bufs=1) as cmx,
                    tc.tile_pool(name="psb", bufs=3, space="PSUM") as psb,
                    tc.tile_pool(name="wkb", bufs=3) as wkb,
                ):
                    # ---- pass A (term_col), pipelined over column groups ----
                    colmax = [cmx.tile([128, B], BF16, tag=f"colmax{m}",
                                       name=f"colmax{m}") for m in range(NMC_A)]
                    for cg, (off, w, j0, nj) in enumerate(CGROUPS):
                        for m in range(NMC_A):
                            for rr in range(NCORES):
                                pa = psa.tile([128, 512], F32, tag="pa")
                                for k in range(2):
                                    mm = nc.tensor.matmul(
                                        pa[:, 0:w],
                                        lhsT=cfT[k][:, m * 128:(m + 1) * 128],
                                        rhs=wfg[k][cg][:, rr * w:(rr + 1) * w],
                                        start=(k == 0), stop=(k == 1))
                                nc.vector.reduce_max(
                                    colmax[m][:, rr * BL + j0: rr * BL + j0 + nj],
                                    pa[:, 0:w].rearrange("p (j l) -> p j l", l=L),
                                    axis=AX.X)
                    for m in range(NMC_A):
                        nc.tensor.matmul(
                            term_col[:],
                            lhsT=eones_sb[:],
                            rhs=colmax[m][:],
                            start=(m == 0), stop=(m == NMC_A - 1))

                    # ---- pass B (term_row) ----
                    cc = 0
                    for cg, (off, w, j0, nj) in enumerate(CGROUPS):
                        for rr in range(NCORES):
                            h = rr // 4
                            cbase = 0
                            for ci, P in enumerate(CG_BCHUNKS[cg]):
                                pbn = []
                                for n in range(2):
                                    pb = psb.tile([128, 512], F32, tag="pb",
                                                  name=f"pb{n}")
                                    for k in range(2):
                                        nc.tensor.matmul(
                                            pb[0:P, :],
                                            lhsT=wfg[k][cg][:, rr * w + cbase:
                                                            rr * w + cbase + P],
                                            rhs=cfT[k][:, n * 512:(n + 1) * 512],
                                            start=(k == 0), stop=(k == 1))
                                    pbn.append(pb)
                                pbs0 = wkb.tile([128, 512], BF16, tag="pbs0")
                                pbs1 = wkb.tile([128, 512], BF16, tag="pbs1")
                                nc.scalar.copy(pbs0[0:P, :], pbn[0][0:P, :])
                                nc.scalar.copy(pbs1[0:P, :], pbn[1][0:P, :])
                                t1 = wkb.tile([128, 512], BF16, tag="t1")
                                nc.vector.tensor_tensor(
                                    t1[0:P, :], pbs0[0:P, :],
                                    pbs1[0:P, :], op=ALU.max)
                                t2 = wkb.tile([128, 256], BF16, tag="t2")
                                nc.vector.tensor_tensor(
                                    t2[0:P, :], t1[0:P, 0:256], t1[0:P, 256:512],
                                    op=ALU.max)
                                t3 = wkb.tile([128, 128], BF16, tag="t3")
                                nc.vector.tensor_tensor(
                                    t3[0:P, :], t2[0:P, 0:128], t2[0:P, 128:256],
                                    op=ALU.max)
                                t4 = wkb.tile([128, 64], BF16, tag="t4")
                                nc.vector.tensor_tensor(
                                    t4[0:P, :], t3[0:P, 0:64], t3[0:P, 64:128],
                                    op=ALU.max)
                                rmt = wkb.tile([128, BL], BF16, tag="rmt")
                                nc.vector.tensor_tensor(
                                    rmt[0:P, :], t4[0:P, 0:32], t4[0:P, 32:64],
                                    op=ALU.max)
                                nc.tensor.matmul(
                                    term_row[h],
                                    lhsT=emask_sb[0:P, cc * 128:(cc + 1) * 128],
                                    rhs=rmt[0:P, :],
                                    start=(cg == 0 and rr % 4 == 0 and ci == 0),
                                    stop=(cg == 2 and rr % 4 == 3 and ci == 2))
                                cbase += P
                                cc += 1

                if dbg3 is not None:
                    with tc.tile_pool(name="dbgp3", bufs=1) as dbgp3:
                        dcp3 = dbgp3.tile([128, 320], F32, tag="dcp3")
                        nc.vector.tensor_copy(dcp3[0:BL, 0:B], term_col[:]); nc.vector.tensor_copy(dcp3[:, B:B+BL], term_row[0][:]); nc.vector.tensor_copy(dcp3[:, B+BL:B+2*BL], term_row[1][:])
                        nc.sync.dma_start(dbg3[:], dcp3[:])

                # ---- stage 6: sim rows = term_col + term_row^T ----
                with (
                    tc.tile_pool(name="ps6", bufs=2, space="PSUM") as ps6,
                    tc.tile_pool(name="wk6", bufs=2) as wk6,
                ):
                    for h in range(2):
                        trs = wk6.tile([128, BL], F32, tag="trs")
                        nc.scalar.copy(trs[:], term_row[h])
                        ptt = ps6.tile([BL, 128], F32, tag="ptt")
                        nc.tensor.transpose(ptt[:], trs[:], ident_sb[:])
                        tts = wk6.tile([BL, 128], F32, tag="tts")
                        nc.scalar.copy(tts[:], ptt[:])
                        nc.vector.tensor_tensor(
                            sim_sb[:, h * 128:(h + 1) * 128],
                            term_col[0:BL, h * 128:(h + 1) * 128],
                            tts[:], op=ALU.add)

                # ---- stage 7: AllGather sim + loss ----
                ag2_in = dram.tile([BL, B], F32)
                ag2_out = dram.tile([B, B], F32)
                nc.sync.dma_start(ag2_in[:], sim_sb[:])
                nc.gpsimd.collective_compute(
                    "AllGather", ALU.bypass,
                    ins=[ag2_in.opt()], outs=[ag2_out.opt()],
                    replica_groups=[list(range(NCORES))])
                if dbg is not None:
                    nc.sync.dma_start(dbg[:], ag2_out[:])

                with (
                    tc.tile_pool(name="ps7", bufs=2, space="PSUM") as ps7,
                    tc.tile_pool(name="wk7", bufs=1) as wk7,
                ):
                    simf = []
                    for t in range(2):
                        sf = wk7.tile([128, B], F32, tag=f"simf{t}",
                                      name=f"simf{t}")
                        nc.sync.dma_start(sf[:], ag2_out[t * 128:(t + 1) * 128, :])
                        simf.append(sf)

                    loss_acc = wk7.tile([128, 4], F32, tag="loss_acc")

                    def lse_diag_col(src_tile, t, col):
                        nrmax = wk7.tile([128, 1], F32, tag="nrmax")
                        nc.vector.tensor_reduce(nrmax[:], src_tile[:], axis=AX.X,
                                                op=ALU.max, negate=True)
                        escr = wk7.tile([128, B], F32, tag="escr")
                        sume = wk7.tile([128, 1], F32, tag="sume")
                        nc.scalar.activation(escr[:], src_tile[:], ACTF.Exp,
                                             bias=nrmax[:], scale=1.0,
                                             accum_out=sume[:])
                        lg = wk7.tile([128, 1], F32, tag="lg")
                        nc.scalar.activation(lg[:], sume[:], ACTF.Ln)
                        dscr = wk7.tile([128, B], F32, tag="dscr")
                        dg = wk7.tile([128, 1], F32, tag="dg")
                        nc.vector.scalar_tensor_tensor(
                            dscr[:], src_tile[:], 1.0,
                            dmask_sb[:, t * B:(t + 1) * B],
                            op0=ALU.mult, op1=ALU.mult, accum_out=dg[:])
                        nc.vector.scalar_tensor_tensor(
                            loss_acc[:, col:col + 1], nrmax[:], dg[:], lg[:],
                            op0=ALU.add, op1=ALU.subtract)

                    for t in range(2):
                        lse_diag_col(simf[t], t, t)

                    for t in range(2):
                        sT = wk7.tile([128, B], F32, tag=f"simT{t}",
                                      name=f"simT{t}")
                        for u in range(2):
                            pt = ps7.tile([128, 128], F32, tag="pt7")
                            nc.tensor.transpose(
                                pt[:], simf[u][:, t * 128:(t + 1) * 128],
                                ident_sb[:])
                            nc.scalar.copy(sT[:, u * 128:(u + 1) * 128], pt[:])
                        lse_diag_col(sT, t, 2 + t)

                    fin = ps7.tile([1, 4], F32, tag="fin")
                    nc.tensor.matmul(fin[0:1, :], lhsT=ones_sb[:],
                                     rhs=loss_acc[:], start=True, stop=True)
                    red = wk7.tile([1, 1], F32, tag="red")
                    nc.vector.reduce_sum(red[0:1, :], fin[0:1, :], axis=AX.X)
                    osb = wk7.tile([1, 1], F32, tag="osb")
                    nc.scalar.mul(osb[0:1, :], red[0:1, :], -1.0 / (2 * B))
                    nc.sync.dma_start(out[:], osb[0:1, :])

    return nc


def _host_prep(inputs):
    concept_feat = np.ascontiguousarray(np.asarray(inputs["concept_feat"],
                                                   dtype=np.float32))
    text_embeds = np.array(np.asarray(inputs["text_embeds"],
                                      dtype=np.float32), copy=True)
    text_mask = np.asarray(inputs["text_mask"]).astype(np.int32)
    Wc = np.ascontiguousarray(np.asarray(inputs["Wc"], dtype=np.float32))
    bc = np.asarray(inputs["bc"], dtype=np.float32)
    Ww = np.ascontiguousarray(np.asarray(inputs["Ww"], dtype=np.float32))
    bw = np.asarray(inputs["bw"], dtype=np.float32)
    temp = float(np.asarray(inputs["temp_cpt"]))

    # word mask (drop CLS + SEP), valid counts
    m = text_mask.copy()
    m[:, 0] = 0
    sep = (L - 1) - np.argmax(m[:, ::-1] > 0, axis=1)
    m[np.arange(B), sep] = 0
    nw = m.sum(axis=1).astype(np.float32)

    # sanitize invalid text rows with copies of position 1 (always valid)
    for j in range(B):
        inv = m[j] == 0
        text_embeds[j, inv] = text_embeds[j, 1]

    # Eones: partition p of any pass-A M-chunk maps to i = p % BL
    eones = np.zeros((128, BL), dtype=np.float32)
    eones[np.arange(128), np.arange(128) % BL] = 1.0 / (Q * temp)

    # Emask tiles: [NB, 128, 128], chunk order must match pass B loops
    emaskt = np.zeros((NB, 128, 128), dtype=np.float32)
    cc = 0
    for cg, (off, wdt, j0, nj) in enumerate(CGROUPS):
        for rr in range(NCORES):
            cbase = off
            for P in CG_BCHUNKS[cg]:
                for p in range(P):
                    jl = cbase + p
                    j_loc, l = divmod(jl, L)
                    jg = rr * BL + j_loc
                    if m[jg, l]:
                        emaskt[cc, p, jg % 128] = 1.0 / (nw[jg] * temp)
                cbase += P
                cc += 1

    dmask = np.zeros((2, 128, B), dtype=np.float32)
    for t in range(2):
        dmask[t, np.arange(128), t * 128 + np.arange(128)] = 1.0

    shared = {
        "wc": Wc.astype(ml_dtypes.bfloat16), "ww": Ww.astype(ml_dtypes.bfloat16),
        "bcb": np.tile(bc[None, :], (128, 1)).astype(np.float32),
        "bwb": np.tile(bw[None, :], (128, 1)).astype(np.float32),
        "ident": np.eye(128, dtype=np.float32),
        "eones": eones.astype(ml_dtypes.bfloat16),
        "emaskt": emaskt.astype(ml_dtypes.bfloat16),
        "dmask": dmask,
        "ones128": np.ones((128, 1), dtype=np.float32),
    }
    in_maps = []
    for r in range(NCORES):
        im = dict(shared)
        # cfT columns are q-major: row order (q, i)
        im["concept_t"] = np.ascontiguousarray(
            concept_feat[r * BL:(r + 1) * BL].transpose(1, 0, 2)
            .reshape(IQ, VW).T).astype(ml_dtypes.bfloat16)
        im["text_t"] = np.ascontiguousarray(
            text_embeds[r * BL:(r + 1) * BL].reshape(JLL, TW).T
        ).astype(ml_dtypes.bfloat16)
        in_maps.append(im)
    return in_maps


def kernel(**inputs):
    in_maps = _host_prep(inputs)
    if "nc" not in _CACHE:
        _CACHE["nc"] = _build()
    res = run_bass_kernel_spmd(_CACHE["nc"], in_maps,
                               core_ids=list(range(NCORES)))
    return np.float32(res.results[0]["out"][0, 0])


# revision 26
# speedup vs baseline: 1.4015x; 1.1437x over previous
"""ALBEF concept-text contrastive loss on 8 TRN2 NeuronCores.

Key algebraic facts used (verified vs the jax reference to 3e-8):
  * sim_t2i == sim_i2t.T exactly, so only one [B, B] similarity matrix is
    needed; the loss is the mean of row- and column- log-softmax diagonals.
  * sim_i2t[i,j] = term_col + term_row, with S[i,j,q,l] = cf_hat[i,q]. wf_hat[j,l]:
      term_col = sum_q  max_{valid l} S / (Q * temp)
      term_row = sum_{valid l} max_q S / (nw[j] * temp)
  * Invalid text positions (CLS/SEP/pad) are replaced on the host by copies of
    position 1 (always valid), which makes max over ALL l == max over valid l.
    The masked l-sum is folded into a small block-diagonal "Emask" matmul whose
    weights are mask[j,l]/(nw[j]*temp); the q-sum is an "Eones" matmul.

Per-core dataflow (core r owns batch rows r*32:(r+1)*32):
  stage 1/2: project+normalize local concept/text shards, PE-transpose to
    [D, rows] layout (cfT columns are q-major: col = q*32 + i).
  stage 3: AllGather of projected text features, pipelined in 3 column chunks
    so pass A/B compute starts when the first chunk lands.
  pass A: S chunks [part=(q,i), free=(j,l)]; DVE segmented max over l ->
    colmax; Eones matmul accumulates sum_q into term_col (PSUM).
  pass B: S chunks [part=(j,l), free=(q,i) q-major]; ACT copies PSUM->SBUF
    bf16; GpSimd computes max over q by 5 contiguous halving tensor-max ops;
    Emask matmul accumulates the masked l-sum into term_row (PSUM).
  stage 6: sim rows [32, 256] = term_col + term_row^T.
  stage 7: AllGather sim; per-core log-softmax loss (row + col) -> scalar.
"""

import ml_dtypes
import numpy as np

import concourse.bass as bass
import concourse.bacc as bacc
import concourse.mybir as mybir
import concourse.tile as tile
from concourse.bass_utils import run_bass_kernel_spmd

F32 = mybir.dt.float32
BF16 = mybir.dt.bfloat16
AX = mybir.AxisListType
ALU = mybir.AluOpType
ACTF = mybir.ActivationFunctionType

B, Q, L, VW, TW, D = 256, 32, 40, 768, 768, 256
NCORES = 8
BL = B // NCORES            # 32 local batch rows
IQ = BL * Q                 # 1024 local (q,i) rows
JLL = BL * L                # 1280 local (j,l) rows
JL = B * L                  # 10240 global (j,l)
KC = VW // 128              # 6 contraction chunks for projection
NMC_A = IQ // 128           # 8 M-chunks in pass A
# column groups (for AG pipelining + pass chunking): (offset, width, j0, nj)
CGROUPS = [(0, 480, 0, 12), (480, 480, 12, 12), (960, 320, 24, 8)]
# pass B chunk widths within each column group
CG_BCHUNKS = {0: [120, 120, 120, 120], 1: [120, 120, 120, 120], 2: [120, 120, 80]}
NB = sum(len(v) for v in CG_BCHUNKS.values()) * NCORES  # 88

_CACHE = {}


def _build():
    nc = _build_graph()
    nc.compile()
    return nc


def _build_graph():
    nc = bacc.Bacc("TRN2", target_bir_lowering=False, debug=False,
                   num_devices=NCORES)

    concept_t = nc.dram_tensor("concept_t", [VW, IQ], BF16, kind="ExternalInput")
    text_t = nc.dram_tensor("text_t", [TW, JLL], BF16, kind="ExternalInput")
    wc = nc.dram_tensor("wc", [VW, D], BF16, kind="ExternalInput")
    ww = nc.dram_tensor("ww", [TW, D], BF16, kind="ExternalInput")
    bcb = nc.dram_tensor("bcb", [128, D], F32, kind="ExternalInput")
    bwb = nc.dram_tensor("bwb", [128, D], F32, kind="ExternalInput")
    ident = nc.dram_tensor("ident", [128, 128], F32, kind="ExternalInput")
    eones = nc.dram_tensor("eones", [128, NMC_A * BL], BF16, kind="ExternalInput")
    emaskt = nc.dram_tensor("emaskt", [NB, 128, 128], BF16, kind="ExternalInput")
    dmask = nc.dram_tensor("dmask", [2, 128, B], F32, kind="ExternalInput")
    ones128 = nc.dram_tensor("ones128", [128, 1], F32, kind="ExternalInput")

    out = nc.dram_tensor("out", [1, 1], F32, kind="ExternalOutput")
    import os
    dbg = None
    if os.environ.get("KDBG"):
        dbg = nc.dram_tensor("dbg", [B, B], F32, kind="ExternalOutput")
    dbg3 = None
    if os.environ.get("KDBG3"):
        dbg3 = nc.dram_tensor("dbg3", [128, 320], F32, kind="ExternalOutput")
    dbg2 = None
    if os.environ.get("KDBG2"):
        dbg2 = nc.dram_tensor("dbg2", [128, NCORES * 480], F32, kind="ExternalOutput")

    with tile.TileContext(nc) as tc:
        with (
            tc.tile_pool(name="cst", bufs=1) as cst,
            tc.tile_pool(name="feat", bufs=1) as feat,
            tc.tile_pool(name="dram", bufs=1, space="DRAM") as dram,
        ):
            # ---- persistent SBUF tiles ----
            cfT = [feat.tile([128, IQ], BF16, tag=f"cfT{k}", name=f"cfT{k}")
                   for k in range(2)]
            wfl = [feat.tile([128, JLL], BF16, tag=f"wfl{k}", name=f"wfl{k}")
                   for k in range(2)]
            sim_sb = feat.tile([BL, B], F32, tag="sim_sb")

            emask_sb = cst.tile([128, NB * 128], BF16, tag="emask_sb")
            eones_sb = cst.tile([128, NMC_A * BL], BF16, tag="eones_sb")
            ident_sb = cst.tile([128, 128], F32, tag="ident_sb")
            bcb_sb = cst.tile([128, D], F32, tag="bcb_sb")
            bwb_sb = cst.tile([128, D], F32, tag="bwb_sb")
            dmask_sb = cst.tile([128, 2 * B], F32, tag="dmask_sb")
            ones_sb = cst.tile([128, 1], F32, tag="ones_sb")

            nc.scalar.dma_start(emask_sb[:].rearrange("p (c m) -> p c m", m=128),
                                emaskt[:].rearrange("c p m -> p c m"))
            nc.scalar.dma_start(eones_sb[:], eones[:])
            nc.scalar.dma_start(ident_sb[:], ident[:])
            nc.scalar.dma_start(bcb_sb[:], bcb[:])
            nc.scalar.dma_start(bwb_sb[:], bwb[:])
            nc.scalar.dma_start(dmask_sb[:].rearrange("p (t j) -> p t j", j=B),
                                dmask[:].rearrange("t p j -> p t j"))
            nc.scalar.dma_start(ones_sb[:], ones128[:])

            # ---- stages 1-5 under the feat2 (wfg) pool ----
            with tc.tile_pool(name="feat2", bufs=1) as feat2:
                wfg = [[feat2.tile([128, NCORES * CGROUPS[cg][1]], BF16,
                                   tag=f"wfg{k}_{cg}", name=f"wfg{k}_{cg}")
                        for cg in range(3)]
                       for k in range(2)]
                rmt_all = feat2.tile([128, NB * BL], BF16, tag="rmt_all")

                def issue_ag(cg):
                    off, w, _, _ = CGROUPS[cg]
                    ag_in = dram.tile([2 * 128, w], BF16, name=f"ag_in{cg}")
                    ag_out = dram.tile([NCORES * 2 * 128, w], BF16,
                                       name=f"ag_out{cg}")
                    for k in range(2):
                        nc.sync.dma_start(ag_in[k * 128:(k + 1) * 128, :],
                                          wfl[k][:, off:off + w])
                    nc.gpsimd.collective_compute(
                        "AllGather", ALU.bypass,
                        ins=[ag_in.opt()], outs=[ag_out.opt()],
                        replica_groups=[list(range(NCORES))])
                    for rr in range(NCORES):
                        for k in range(2):
                            nc.sync.dma_start(
                                wfg[k][cg][:, rr * w:(rr + 1) * w],
                                ag_out[rr * 256 + k * 128:
                                       rr * 256 + (k + 1) * 128, :])

                # ---- stage 1/2: projections + l2norm + transpose ----
                with (
                    tc.tile_pool(name="pin", bufs=1) as pin,
                    tc.tile_pool(name="ps2", bufs=2, space="PSUM") as ps2,
                    tc.tile_pool(name="wk2", bufs=2) as wk2,
                ):
                    cin = pin.tile([128, KC * IQ], BF16, tag="cin")
                    tin = pin.tile([128, KC * JLL], BF16, tag="tin")
                    wcs = pin.tile([128, KC * D], BF16, tag="wcs")
                    wws = pin.tile([128, KC * D], BF16, tag="wws")
                    for k in range(KC):
                        nc.sync.dma_start(tin[:, k * JLL:(k + 1) * JLL],
                                          text_t[k * 128:(k + 1) * 128, :])
                        nc.sync.dma_start(cin[:, k * IQ:(k + 1) * IQ],
                                          concept_t[k * 128:(k + 1) * 128, :])
                        nc.sync.dma_start(wcs[:, k * D:(k + 1) * D],
                                          wc[k * 128:(k + 1) * 128, :])
                        nc.sync.dma_start(wws[:, k * D:(k + 1) * D],
                                          ww[k * 128:(k + 1) * 128, :])

                    def project(src, width, w_sb, bias_sb, dstT, after_m=None):
                        for m in range(width // 128):
                            pp = ps2.tile([128, D], F32, tag="pp")
                            for k in range(KC):
                                nc.tensor.matmul(
                                    pp[:],
                                    lhsT=src[:, k * width + m * 128:
                                             k * width + (m + 1) * 128],
                                    rhs=w_sb[:, k * D:(k + 1) * D],
                                    start=(k == 0), stop=(k == KC - 1))
                            t_sb = wk2.tile([128, D], F32, tag="t_sb")
                            nc.vector.scalar_tensor_tensor(
                                t_sb[:], pp[:], 1.0, bias_sb[:],
                                op0=ALU.mult, op1=ALU.add)
                            sq = wk2.tile([128, D], F32, tag="sq")
                            ss = wk2.tile([128, 1], F32, tag="ss")
                            nc.scalar.activation(sq[:], t_sb[:], ACTF.Square,
                                                 accum_out=ss[:])
                            ssq = wk2.tile([128, 1], F32, tag="ssq")
                            nc.scalar.sqrt(ssq[:], ss[:])
                            rn = wk2.tile([128, 1], F32, tag="rn")
                            nc.vector.reciprocal(rn[:], ssq[:])
                            nsb = wk2.tile([128, D], F32, tag="nsb")
                            nc.vector.tensor_scalar_mul(nsb[:], t_sb[:], rn[:])
                            for kk in range(2):
                                ptr = ps2.tile([128, 128], F32, tag="ptr")
                                nc.tensor.transpose(
                                    ptr[:], nsb[:, kk * 128:(kk + 1) * 128],
                                    ident_sb[:])
                                nc.scalar.copy(
                                    dstT[kk][:, m * 128:(m + 1) * 128], ptr[:])
                            if after_m is not None and m in after_m:
                                after_m[m]()

                    project(tin, JLL, wws, bwb_sb, wfl,
                            after_m={3: lambda: issue_ag(0),
                                     7: lambda: issue_ag(1),
                                     9: lambda: issue_ag(2)})
                    project(cin, IQ, wcs, bcb_sb, cfT)

                with (
                    tc.tile_pool(name="psa", bufs=2, space="PSUM") as psa,
                    tc.tile_pool(name="cmx", bufs=1) as cmx,
                    tc.tile_pool(name="psb", bufs=2, space="PSUM") as psb,
                ):
                    # ---- pass A (term_col), pipelined over column groups ----
                    colmax = [cmx.tile([128, B], BF16, tag=f"colmax{m}",
                                       name=f"colmax{m}") for m in range(NMC_A)]
                    for cg, (off, w, j0, nj) in enumerate(CGROUPS):
                        for m in range(NMC_A):
                            for rp in range(NCORES // 2):
                                pa = psa.tile([128, 1024], F32, tag="pa")
                                for k in range(2):
                                    for half in range(2):
                                        rr = 2 * rp + half
                                        mm = nc.tensor.matmul(
                                            pa[:, half * 512: half * 512 + w],
                                            lhsT=cfT[k][:, m * 128:(m + 1) * 128],
                                            rhs=wfg[k][cg][:, rr * w:(rr + 1) * w],
                                            start=(k == 0), stop=(k == 1))
                                        if half == 1:
                                            mm.ldweights = False
                                cmv = colmax[m][:].rearrange(
                                    "p (r c) -> p r c", c=BL)[
                                        :, 2 * rp:2 * rp + 2, j0:j0 + nj]
                                pav = pa[:].rearrange(
                                    "p (r x) -> p r x", x=512)[:, :, 0:w].rearrange(
                                    "p r (j l) -> p r j l", l=L)
                                nc.vector.reduce_max(cmv, pav, axis=AX.X)

                    # ---- pass B (term_row) ----
                    cc = 0
                    for cg, (off, w, j0, nj) in enumerate(CGROUPS):
                        for rr in range(NCORES):
                            cbase = 0
                            for ci, P in enumerate(CG_BCHUNKS[cg]):
                                pb = psb.tile([128, 1024], F32, tag="pb")
                                for k in range(2):
                                    for n in range(2):
                                        mm = nc.tensor.matmul(
                                            pb[0:P, n * 512:(n + 1) * 512],
                                            lhsT=wfg[k][cg][:, rr * w + cbase:
                                                            rr * w + cbase + P],
                                            rhs=cfT[k][:, n * 512:(n + 1) * 512],
                                            start=(k == 0), stop=(k == 1))
                                        if n == 1:
                                            mm.ldweights = False
                                nc.vector.reduce_max(
                                    rmt_all[0:P, cc * BL:(cc + 1) * BL],
                                    pb[0:P, :].rearrange("p (i q) -> p i q", q=Q),
                                    axis=AX.X)
                                cbase += P
                                cc += 1

                # ---- tail: deferred E-matmuls into PSUM accumulators ----
                with tc.tile_pool(name="pterm", bufs=1, space="PSUM") as pterm:
                  term_col = pterm.tile([BL, B], F32, tag="tcol")
                  term_row = [pterm.tile([128, BL], F32, tag=f"trow{h}",
                                         name=f"trow{h}") for h in range(2)]
                  for m in range(NMC_A):
                      nc.tensor.matmul(
                          term_col[:],
                          lhsT=eones_sb[:, m * BL:(m + 1) * BL],
                          rhs=colmax[m][:],
                          start=(m == 0), stop=(m == NMC_A - 1))
                  cc = 0
                  for cg in range(3):
                      for rr in range(NCORES):
                          h = rr // 4
                          for ci, P in enumerate(CG_BCHUNKS[cg]):
                              nc.tensor.matmul(
                                  term_row[h],
                                  lhsT=emask_sb[0:P, cc * 128:(cc + 1) * 128],
                                  rhs=rmt_all[0:P, cc * BL:(cc + 1) * BL],
                                  start=(cg == 0 and rr % 4 == 0 and ci == 0),
                                  stop=(cg == 2 and rr % 4 == 3 and ci == 2))
                              cc += 1

                  # ---- stage 6: sim rows = term_col + term_row^T ----
                  with (
                      tc.tile_pool(name="ps6", bufs=2, space="PSUM") as ps6,
                      tc.tile_pool(name="wk6", bufs=2) as wk6,
                  ):
                    for h in range(2):
                        trs = wk6.tile([128, BL], F32, tag="trs")
                        nc.scalar.copy(trs[:], term_row[h][:])
                        ptt = ps6.tile([BL, 128], F32, tag="ptt")
                        nc.tensor.transpose(ptt[:], trs[:], ident_sb[:])
                        tts = wk6.tile([BL, 128], F32, tag="tts")
                        nc.scalar.copy(tts[:], ptt[:])
                        nc.vector.tensor_tensor(
                            sim_sb[:, h * 128:(h + 1) * 128],
                            term_col[0:BL, h * 128:(h + 1) * 128],
                            tts[:], op=ALU.add)

                # ---- stage 7: AllGather sim + loss ----
                ag2_in = dram.tile([BL, B], F32)
                ag2_out = dram.tile([B, B], F32)
                nc.sync.dma_start(ag2_in[:], sim_sb[:])
                nc.gpsimd.collective_compute(
                    "AllGather", ALU.bypass,
                    ins=[ag2_in.opt()], outs=[ag2_out.opt()],
                    replica_groups=[list(range(NCORES))])
                if dbg is not None:
                    nc.sync.dma_start(dbg[:], ag2_out[:])

                with (
                    tc.tile_pool(name="ps7", bufs=2, space="PSUM") as ps7,
                    tc.tile_pool(name="wk7", bufs=1) as wk7,
                ):
                    simf = []
                    for t in range(2):
                        sf = wk7.tile([128, B], F32, tag=f"simf{t}",
                                      name=f"simf{t}")
                        nc.sync.dma_start(sf[:], ag2_out[t * 128:(t + 1) * 128, :])
                        simf.append(sf)

                    loss_acc = wk7.tile([128, 4], F32, tag="loss_acc")

                    def lse_diag_col(src_tile, t, col):
                        nrmax = wk7.tile([128, 1], F32, tag="nrmax")
                        nc.vector.tensor_reduce(nrmax[:], src_tile[:], axis=AX.X,
                                                op=ALU.max, negate=True)
                        escr = wk7.tile([128, B], F32, tag="escr")
                        sume = wk7.tile([128, 1], F32, tag="sume")
                        nc.scalar.activation(escr[:], src_tile[:], ACTF.Exp,
                                             bias=nrmax[:], scale=1.0,
                                             accum_out=sume[:])
                        lg = wk7.tile([128, 1], F32, tag="lg")
                        nc.scalar.activation(lg[:], sume[:], ACTF.Ln)
                        dscr = wk7.tile([128, B], F32, tag="dscr")
                        dg = wk7.tile([128, 1], F32, tag="dg")
                        nc.vector.scalar_tensor_tensor(
                            dscr[:], src_tile[:], 1.0,
                            dmask_sb[:, t * B:(t + 1) * B],
                            op0=ALU.mult, op1=ALU.mult, accum_out=dg[:])
                        nc.vector.scalar_tensor_tensor(
                            loss_acc[:, col:col + 1], nrmax[:], dg[:], lg[:],
                            op0=ALU.add, op1=ALU.subtract)

                    for t in range(2):
                        lse_diag_col(simf[t], t, t)

                    for t in range(2):
                        sT = wk7.tile([128, B], F32, tag=f"simT{t}",
                                      name=f"simT{t}")
                        for u in range(2):
                            pt = ps7.tile([128, 128], F32, tag="pt7")
                            nc.tensor.transpose(
                                pt[:], simf[u][:, t * 128:(t + 1) * 128],
                                ident_sb[:])
                            nc.scalar.copy(sT[:, u * 128:(u + 1) * 128], pt[:])
                        lse_diag_col(sT, t, 2 + t)

                    fin = ps7.tile([1, 4], F32, tag="fin")
                    nc.tensor.matmul(fin[0:1, :], lhsT=ones_sb[:],
                                     rhs=loss_acc[:], start=True, stop=True)
                    red = wk7.tile([1, 1], F32, tag="red")
                    nc.vector.reduce_sum(red[0:1, :], fin[0:1, :], axis=AX.X)
                    osb = wk7.tile([1, 1], F32, tag="osb")
                    nc.scalar.mul(osb[0:1, :], red[0:1, :], -1.0 / (2 * B))
                    nc.sync.dma_start(out[:], osb[0:1, :])

    return nc


def _host_prep(inputs):
    concept_feat = np.ascontiguousarray(np.asarray(inputs["concept_feat"],
                                                   dtype=np.float32))
    text_embeds = np.array(np.asarray(inputs["text_embeds"],
                                      dtype=np.float32), copy=True)
    text_mask = np.asarray(inputs["text_mask"]).astype(np.int32)
    Wc = np.ascontiguousarray(np.asarray(inputs["Wc"], dtype=np.float32))
    bc = np.asarray(inputs["bc"], dtype=np.float32)
    Ww = np.ascontiguousarray(np.asarray(inputs["Ww"], dtype=np.float32))
    bw = np.asarray(inputs["bw"], dtype=np.float32)
    temp = float(np.asarray(inputs["temp_cpt"]))

    # word mask (drop CLS + SEP), valid counts
    m = text_mask.copy()
    m[:, 0] = 0
    sep = (L - 1) - np.argmax(m[:, ::-1] > 0, axis=1)
    m[np.arange(B), sep] = 0
    nw = m.sum(axis=1).astype(np.float32)

    # sanitize invalid text rows with copies of position 1 (always valid)
    for j in range(B):
        inv = m[j] == 0
        text_embeds[j, inv] = text_embeds[j, 1]

    # Eones: [128, NMC_A*BL]; pass-A chunk m covers (i,q) rows 128m..128m+127
    eones = np.zeros((128, NMC_A * BL), dtype=np.float32)
    for mm in range(NMC_A):
        for p in range(128):
            ig = 4 * mm + p // Q
            eones[p, mm * BL + ig] = 1.0 / (Q * temp)

    # Emask tiles: [NB, 128, 128], chunk order must match pass B loops
    emaskt = np.zeros((NB, 128, 128), dtype=np.float32)
    cc = 0
    for cg, (off, wdt, j0, nj) in enumerate(CGROUPS):
        for rr in range(NCORES):
            cbase = off
            for P in CG_BCHUNKS[cg]:
                for p in range(P):
                    jl = cbase + p
                    j_loc, l = divmod(jl, L)
                    jg = rr * BL + j_loc
                    if m[jg, l]:
                        emaskt[cc, p, jg % 128] = 1.0 / (nw[jg] * temp)
                cbase += P
                cc += 1

    dmask = np.zeros((2, 128, B), dtype=np.float32)
    for t in range(2):
        dmask[t, np.arange(128), t * 128 + np.arange(128)] = 1.0

    shared = {
        "wc": Wc.astype(ml_dtypes.bfloat16), "ww": Ww.astype(ml_dtypes.bfloat16),
        "bcb": np.tile(bc[None, :], (128, 1)).astype(np.float32),
        "bwb": np.tile(bw[None, :], (128, 1)).astype(np.float32),
        "ident": np.eye(128, dtype=np.float32),
        "eones": eones.astype(ml_dtypes.bfloat16),
        "emaskt": emaskt.astype(ml_dtypes.bfloat16),
        "dmask": dmask,
        "ones128": np.ones((128, 1), dtype=np.float32),
    }
    in_maps = []
    for r in range(NCORES):
        im = dict(shared)
        im["concept_t"] = np.ascontiguousarray(
            concept_feat[r * BL:(r + 1) * BL].reshape(IQ, VW).T
        ).astype(ml_dtypes.bfloat16)
        im["text_t"] = np.ascontiguousarray(
            text_embeds[r * BL:(r + 1) * BL].reshape(JLL, TW).T
        ).astype(ml_dtypes.bfloat16)
        in_maps.append(im)
    return in_maps


def kernel(**inputs):
    in_maps = _host_prep(inputs)
    if "nc" not in _CACHE:
        _CACHE["nc"] = _build()
    res = run_bass_kernel_spmd(_CACHE["nc"], in_maps,
                               core_ids=list(range(NCORES)))
    return np.float32(res.results[0]["out"][0, 0])
